# revision 6
# baseline (speedup 1.0000x reference)
"""Trainium2 Bass kernel for causal self-attention with RoPE.

Problem: y = CausalSelfAttention(x) with
  B, T, C, H = 4, 2048, 1024, 16; D = 64; RoPE base 10000; no 1/sqrt(D) scale.

Sharding: Megatron-style tensor parallel over heads. 8 cores, 2 heads each.
Each core computes qkv for its 2 heads (columns of W_qkv), runs attention for
its (b, head) pairs, and multiplies by its 128 rows of W_out, producing a
partial (B*T, C) output. The host sums the 8 partials and adds b_out.

Per-core device pipeline:
  phase 1: qT/kT/vT [128, B*T] (2 heads stacked on partitions) via PE matmuls
           with W chunks stationary; RoPE on q/k via a +-1 permutation matmul
           (rotate_half) + DVE mul/adds; v transposed back to [tokens, 128]
           via PE transpose, stored chunked with an interleaved ones column.
  phase 2: per (b, head): S^T = kT_chunk^T @ qT_block on PE (s on partitions),
           exp on ACT (no max subtraction: |score| <~ 60, fp32 exp can't
           overflow), causal mask via 0/1 multiply, P^T @ [v | 1] accumulated
           on PE -> O^T rows + denominator row, normalize via reciprocal +
           K=1 broadcast matmul.
  phase 3: out_partial[tokens, C] = O^T.T @ W_out_rows on PE.

Matmul dtype strategy (RMODE):
  "all":  every matmul in float32r (1 cy/row on PE vs 4 for float32;
          ~12-bit mantissa operand rounding, fp32 accumulate).
  "qk32": q/k projection + scores matmul in float32 (full precision on the
          exp-amplified path), everything else float32r.
  "fp32": everything float32.
"""

import numpy as np
from contextlib import ExitStack

import concourse.mybir as mybir
import concourse.tile as tile
from concourse import bacc
from concourse.bass_utils import run_bass_kernel_spmd
from concourse.masks import make_identity

F32 = mybir.dt.float32
F32R = mybir.dt.float32r
AF = mybir.ActivationFunctionType

C = 1024
H = 16
D = 64
N_CORES = 8
HPC = H // N_CORES          # heads per core = 2
ROPE_BASE = 10000.0
KC = C // 128               # contraction chunks for the qkv projection = 8

RMODE = "all"


def build_program(B, T, use_qk_bias, use_v_bias, rmode=RMODE, n_cores=N_CORES):
    TOK = B * T
    NB = TOK // 512           # 512-token blocks
    NCHUNK = TOK // 128       # 128-token chunks (v storage)
    QB = T // 512             # q-blocks per sequence
    CS = T // 512             # distinct 512-col blocks of the rope tables

    # rmode: "all" | "qk32" | "fp32", or a 3-tuple of dtypes
    # (d_qkproj, d_attn, d_out) for bisection.
    if isinstance(rmode, tuple):
        QKD, AD, OD = rmode
    elif rmode == "all":
        QKD = AD = OD = F32R
    elif rmode == "qk32":
        QKD, AD, OD = F32, F32R, F32R
    else:
        QKD = AD = OD = F32

    nc = bacc.Bacc("TRN2", target_bir_lowering=False, debug=False,
                   num_devices=n_cores)

    xT = nc.dram_tensor("xT", [C, TOK], QKD, kind="ExternalInput").ap()
    wq = nc.dram_tensor("wq", [C, 128], QKD, kind="ExternalInput").ap()
    wk = nc.dram_tensor("wk", [C, 128], QKD, kind="ExternalInput").ap()
    wv = nc.dram_tensor("wv", [C, 128], AD, kind="ExternalInput").ap()
    wo = nc.dram_tensor("wo", [128, C], OD, kind="ExternalInput").ap()
    cosT = nc.dram_tensor("cosT", [128, T], F32, kind="ExternalInput").ap()
    sinT = nc.dram_tensor("sinT", [128, T], F32, kind="ExternalInput").ap()
    msk = nc.dram_tensor("msk", [128, 2048], F32, kind="ExternalInput").ap()
    rot = nc.dram_tensor("rot", [128, 128], AD, kind="ExternalInput").ap()
    if use_qk_bias:
        bq = nc.dram_tensor("bq", [128, 1], F32, kind="ExternalInput").ap()
        bk = nc.dram_tensor("bk", [128, 1], F32, kind="ExternalInput").ap()
    if use_v_bias:
        bv = nc.dram_tensor("bv", [128, 1], F32, kind="ExternalInput").ap()
    outp = nc.dram_tensor("outp", [TOK, C], F32, kind="ExternalOutput").ap()

    with tile.TileContext(nc) as tc:
        with ExitStack() as res:  # tensors resident through phases 1+2
            persist = res.enter_context(tc.tile_pool(name="persist", bufs=1))
            qT = persist.tile([128, TOK], AD)
            kT = persist.tile([128, TOK], AD)
            vsb = persist.tile([128, NCHUNK * 130], AD)
            ones_sb = persist.tile([1, 64], AD)
            nc.vector.memset(ones_sb[:].bitcast(F32) if AD == F32R else ones_sb[:], 1.0)

            with tc.tile_pool(name="ot", bufs=1) as otpool:
                OT = otpool.tile([128, TOK], OD)

                # ---------------- phase 1: qkv projection + RoPE ----------
                with ExitStack() as p1:
                    cpool = p1.enter_context(tc.tile_pool(name="p1c", bufs=1))
                    wq_sb = cpool.tile([128, C], QKD)
                    wk_sb = cpool.tile([128, C], QKD)
                    wv_sb = cpool.tile([128, C], AD)
                    cos_sb = cpool.tile([128, T], F32)
                    sin_sb = cpool.tile([128, T], F32)
                    rot_sb = cpool.tile([128, 128], AD)
                    ident = cpool.tile([128, 128], F32)
                    make_identity(nc, ident[:])
                    for k in range(KC):
                        nc.sync.dma_start(wq_sb[:, k * 128:(k + 1) * 128],
                                          wq[k * 128:(k + 1) * 128, :])
                        nc.sync.dma_start(wk_sb[:, k * 128:(k + 1) * 128],
                                          wk[k * 128:(k + 1) * 128, :])
                        nc.sync.dma_start(wv_sb[:, k * 128:(k + 1) * 128],
                                          wv[k * 128:(k + 1) * 128, :])
                    nc.sync.dma_start(cos_sb[:], cosT[:])
                    nc.sync.dma_start(sin_sb[:], sinT[:])
                    nc.sync.dma_start(rot_sb[:], rot[:])
                    if use_qk_bias:
                        bq_sb = cpool.tile([128, 1], F32)
                        bk_sb = cpool.tile([128, 1], F32)
                        nc.sync.dma_start(bq_sb[:], bq[:])
                        nc.sync.dma_start(bk_sb[:], bk[:])
                    if use_v_bias:
                        bv_sb = cpool.tile([128, 1], F32)
                        nc.sync.dma_start(bv_sb[:], bv[:])

                    xpool = p1.enter_context(tc.tile_pool(name="xp", bufs=8))
                    xrpool = (xpool if QKD == AD else
                              p1.enter_context(tc.tile_pool(name="xrp", bufs=10)))
                    qkpsum = p1.enter_context(
                        tc.tile_pool(name="qkp", bufs=2, space="PSUM"))
                    rotpsum = p1.enter_context(
                        tc.tile_pool(name="rotp", bufs=1, space="PSUM"))
                    vpsum = p1.enter_context(
                        tc.tile_pool(name="vp", bufs=2, space="PSUM"))
                    tppsum = p1.enter_context(
                        tc.tile_pool(name="tpp", bufs=2, space="PSUM"))
                    tmp = p1.enter_context(tc.tile_pool(name="tmp", bufs=6))

                    for nb in range(NB):
                        t512 = slice(nb * 512, (nb + 1) * 512)
                        cs = slice((nb % CS) * 512, (nb % CS) * 512 + 512)
                        xc = []
                        for k in range(KC):
                            t = xpool.tile([128, 512], QKD, tag="xc")
                            nc.sync.dma_start(
                                t[:], xT[k * 128:(k + 1) * 128, t512])
                            xc.append(t)
                        if QKD == AD:
                            xcv = xc
                        else:
                            # round a second copy for the fp32r v projection
                            xcv = []
                            for k in range(KC):
                                t = xrpool.tile([128, 512], AD, tag="xcv")
                                nc.gpsimd.tensor_copy(t[:], xc[k][:])
                                xcv.append(t)
                        for w_sb, b_name, dstT in ((wq_sb, "bq", qT),
                                                   (wk_sb, "bk", kT)):
                            acc = qkpsum.tile([128, 512], F32, tag="acc")
                            for k in range(KC):
                                nc.tensor.matmul(
                                    acc[:], w_sb[:, k * 128:(k + 1) * 128],
                                    xc[k][:], start=(k == 0), stop=(k == KC - 1))
                            raw = tmp.tile([128, 512], AD, tag="ropetmp")
                            if use_qk_bias:
                                b_sb = bq_sb if b_name == "bq" else bk_sb
                                nc.vector.tensor_scalar_add(raw[:], acc[:],
                                                            b_sb[:])
                            else:
                                nc.vector.tensor_copy(raw[:], acc[:])
                            rp = rotpsum.tile([128, 512], F32, tag="rp")
                            nc.tensor.matmul(rp[:], rot_sb[:], raw[:],
                                             start=True, stop=True)
                            t1 = tmp.tile([128, 512], F32, tag="ropetmp")
                            nc.vector.tensor_mul(t1[:], raw[:], cos_sb[:, cs])
                            t2 = tmp.tile([128, 512], F32, tag="ropetmp")
                            nc.vector.tensor_mul(t2[:], rp[:], sin_sb[:, cs])
                            nc.vector.tensor_add(dstT[:, t512], t1[:], t2[:])
                        # v: project transposed (N=512 streams), then PE-
                        # transpose back to [tokens, 128] chunks
                        vacc = vpsum.tile([128, 512], F32, tag="vacc")
                        for k in range(KC):
                            nc.tensor.matmul(
                                vacc[:], wv_sb[:, k * 128:(k + 1) * 128],
                                xcv[k][:], start=(k == 0), stop=(k == KC - 1))
                        vraw = tmp.tile([128, 512], F32, tag="vraw", bufs=2)
                        if use_v_bias:
                            nc.vector.tensor_scalar_add(vraw[:], vacc[:],
                                                        bv_sb[:])
                        else:
                            nc.vector.tensor_copy(vraw[:], vacc[:])
                        for sub in range(4):
                            tp = tppsum.tile([128, 128], F32, tag="tp")
                            nc.tensor.transpose(
                                tp[:], vraw[:, sub * 128:(sub + 1) * 128],
                                ident[:])
                            base = (nb * 4 + sub) * 130
                            nc.vector.tensor_copy(vsb[:, base:base + 64],
                                                  tp[:, 0:64])
                            nc.vector.tensor_copy(vsb[:, base + 65:base + 129],
                                                  tp[:, 64:128])
                            nc.vector.memset(vsb[:, base + 64:base + 65].bitcast(F32) if AD == F32R else vsb[:, base + 64:base + 65], 1.0)
                            nc.vector.memset(vsb[:, base + 129:base + 130].bitcast(F32) if AD == F32R else vsb[:, base + 129:base + 130], 1.0)

                # ---------------- phase 2: causal attention ---------------
                with ExitStack() as p2:
                    mpool = p2.enter_context(tc.tile_pool(name="mp", bufs=1))
                    msk_sb = mpool.tile([128, 2048], F32)
                    nc.sync.dma_start(msk_sb[:], msk[:])
                    spsum = p2.enter_context(
                        tc.tile_pool(name="sp", bufs=3, space="PSUM"))
                    popsum = p2.enter_context(
                        tc.tile_pool(name="pop", bufs=2, space="PSUM"))
                    bcpsum = p2.enter_context(
                        tc.tile_pool(name="bcp", bufs=2, space="PSUM"))
                    ppool = p2.enter_context(tc.tile_pool(name="pp", bufs=3))
                    dpool = p2.enter_context(tc.tile_pool(name="dp", bufs=2))
                    bspool = p2.enter_context(tc.tile_pool(name="bs", bufs=2))

                    for b in range(B):
                        t0 = b * T
                        c0 = b * (T // 128)
                        for hp in range(HPC):
                            hs = slice(hp * 64, (hp + 1) * 64)
                            for qb in range(QB):
                                q512 = slice(t0 + qb * 512, t0 + (qb + 1) * 512)
                                po = popsum.tile([65, 512], F32, tag="po")
                                ns = (qb + 1) * 4
                                for si in range(ns):
                                    S = spsum.tile([128, 512], F32, tag="S")
                                    s0 = t0 + si * 128
                                    nc.tensor.matmul(
                                        S[:], kT[hs, s0:s0 + 128],
                                        qT[hs, q512], start=True, stop=True)
                                    P = ppool.tile([128, 512], AD, tag="P")
                                    nc.scalar.activation(P[:], S[:], AF.Exp)
                                    off = si * 128 - qb * 512
                                    if off >= 0:
                                        vi = off // 128
                                        nc.vector.tensor_mul(
                                            P[:], P[:],
                                            msk_sb[:, vi * 512:(vi + 1) * 512])
                                    vbase = (c0 + si) * 130 + hp * 65
                                    nc.tensor.matmul(
                                        po[:], vsb[:, vbase:vbase + 65], P[:],
                                        start=(si == 0), stop=(si == ns - 1))
                                den32 = dpool.tile([1, 512], F32, tag="den32")
                                nc.vector.reciprocal(den32[:], po[64:65, :])
                                den = dpool.tile([1, 512], AD, tag="den")
                                nc.vector.tensor_copy(den[:], den32[:])
                                bc = bcpsum.tile([64, 512], F32, tag="bc")
                                nc.tensor.matmul(bc[:], ones_sb[:], den[:],
                                                 start=True, stop=True)
                                bcs = bspool.tile([64, 512], F32, tag="bcs")
                                nc.vector.tensor_copy(bcs[:], bc[:])
                                nc.vector.tensor_mul(
                                    OT[hs, q512], po[0:64, :], bcs[:])

                # ---------------- phase 3: output projection --------------
                with ExitStack() as p3:
                    wpool = p3.enter_context(tc.tile_pool(name="wop", bufs=1))
                    wo_sb = wpool.tile([128, C], OD)
                    nc.sync.dma_start(wo_sb[:], wo[:])
                    opsum = p3.enter_context(
                        tc.tile_pool(name="op", bufs=4, space="PSUM"))
                    ostage = p3.enter_context(tc.tile_pool(name="os", bufs=4))
                    for ci in range(NCHUNK):
                        for n2 in range(C // 512):
                            pacc = opsum.tile([128, 512], F32, tag="pacc")
                            nc.tensor.matmul(
                                pacc[:], OT[:, ci * 128:(ci + 1) * 128],
                                wo_sb[:, n2 * 512:(n2 + 1) * 512],
                                start=True, stop=True)
                            osb = ostage.tile([128, 512], F32, tag="osb")
                            if (ci + n2) % 2 == 0:
                                nc.scalar.activation(osb[:], pacc[:], AF.Copy)
                            else:
                                nc.vector.tensor_copy(osb[:], pacc[:])
                            nc.sync.dma_start(
                                outp[ci * 128:(ci + 1) * 128,
                                     n2 * 512:(n2 + 1) * 512], osb[:])

    nc.compile()
    return nc


def make_rope_tables(T, dtype=np.float32):
    j = np.arange(32, dtype=np.float32)
    inv_freq = (1.0 / (ROPE_BASE ** (2.0 * j / D))).astype(np.float32)
    t = np.arange(T, dtype=np.float32)
    freqs = t[None, :] * inv_freq[:, None]          # [32, T]
    half = np.concatenate([freqs, freqs], axis=0)   # [64, T]
    cosT = np.cos(half).astype(dtype)
    sinT = np.sin(half).astype(dtype)
    return (np.concatenate([cosT, cosT], axis=0),   # [128, T] (2 heads)
            np.concatenate([sinT, sinT], axis=0))


def make_rot_matrix():
    """lhsT R [128,128] s.t. (R.T @ x)[m] = rotate_half(x)[m] per 64-row head."""
    R = np.zeros((128, 128), dtype=np.float32)
    for hb in (0, 64):
        for m in range(32):
            R[hb + m + 32, hb + m] = -1.0
            R[hb + m, hb + m + 32] = 1.0
    return R


def make_masks():
    """[128, 4*512]: variant vi valid where s + vi*128 <= q."""
    s = np.arange(128)[:, None]
    q = np.arange(512)[None, :]
    blocks = [(s + vi * 128 <= q).astype(np.float32) for vi in range(4)]
    return np.concatenate(blocks, axis=1)


def prep_in_maps(x, W_qkv, b_qkv, W_out, B, T, use_qk_bias, use_v_bias,
                 n_cores=N_CORES):
    TOK = B * T
    xTm = np.ascontiguousarray(x.reshape(TOK, C).T)
    cosT, sinT = make_rope_tables(T)
    msk = make_masks()
    rot = make_rot_matrix()
    in_maps = []
    for c in range(n_cores):
        h0 = c * HPC
        cols = slice(h0 * D, (h0 + HPC) * D)        # 128 head-dim columns
        m = {
            "xT": xTm,
            "wq": np.ascontiguousarray(W_qkv[:, cols]),
            "wk": np.ascontiguousarray(W_qkv[:, C:][:, cols]),
            "wv": np.ascontiguousarray(W_qkv[:, 2 * C:][:, cols]),
            "wo": np.ascontiguousarray(W_out[cols, :]),
            "cosT": cosT, "sinT": sinT, "msk": msk, "rot": rot,
        }
        if use_qk_bias:
            m["bq"] = np.ascontiguousarray(b_qkv[cols]).reshape(128, 1)
            m["bk"] = np.ascontiguousarray(b_qkv[C:][cols]).reshape(128, 1)
        if use_v_bias:
            m["bv"] = np.ascontiguousarray(b_qkv[2 * C:][cols]).reshape(128, 1)
        in_maps.append(m)
    return in_maps


_CACHE = {}


def _get_program(key):
    if key not in _CACHE:
        B, T, use_qk_bias, use_v_bias = key
        _CACHE[key] = build_program(B, T, use_qk_bias, use_v_bias)
    return _CACHE[key]


def kernel(x, W_qkv, b_qkv, W_out, b_out):
    x = np.asarray(x, dtype=np.float32)
    W_qkv = np.asarray(W_qkv, dtype=np.float32)
    b_qkv = np.asarray(b_qkv, dtype=np.float32)
    W_out = np.asarray(W_out, dtype=np.float32)
    b_out = np.asarray(b_out, dtype=np.float32)
    B, T, _ = x.shape
    use_qk_bias = bool(np.any(b_qkv[:2 * C]))
    use_v_bias = bool(np.any(b_qkv[2 * C:]))
    nc = _get_program((B, T, use_qk_bias, use_v_bias))
    in_maps = prep_in_maps(x, W_qkv, b_qkv, W_out, B, T,
                           use_qk_bias, use_v_bias)
    res = run_bass_kernel_spmd(nc, in_maps, list(range(N_CORES)))
    acc = res.results[0]["outp"].astype(np.float32)
    for c in range(1, N_CORES):
        acc = acc + res.results[c]["outp"]
    acc = acc + b_out[None, :]
    return acc.reshape(B, T, C)


# revision 7
# speedup vs baseline: 1.6780x; 1.6780x over previous
"""Trainium2 Bass kernel for causal self-attention with RoPE.

Problem: y = CausalSelfAttention(x) with
  B, T, C, H = 4, 2048, 1024, 16; D = 64; RoPE base 10000; no 1/sqrt(D) scale.

Sharding: Megatron-style tensor parallel over heads. 8 cores, 2 heads each.
Each core computes qkv for its 2 heads (columns of W_qkv), runs attention for
its (b, head) pairs, and multiplies by its 128 rows of W_out, producing a
partial (B*T, C) output. The host sums the 8 partials and adds b_out.

Per-core device pipeline:
  phase 1: qT/kT/vT [128, B*T] (2 heads stacked on partitions) via PE matmuls
           with W chunks stationary; RoPE on q/k via a +-1 permutation matmul
           (rotate_half) + DVE mul/adds; v transposed back to [tokens, 128]
           via PE transpose, stored chunked with an interleaved ones column.
  phase 2: per (b, head): S^T = kT_chunk^T @ qT_block on PE (s on partitions),
           exp on ACT (no max subtraction: |score| <~ 60, fp32 exp can't
           overflow), causal mask via 0/1 multiply, P^T @ [v | 1] accumulated
           on PE -> O^T rows + denominator row, normalize via reciprocal +
           K=1 broadcast matmul.
  phase 3: out_partial[tokens, C] = O^T.T @ W_out_rows on PE.

Matmul dtype strategy (RMODE):
  "all":  every matmul in float32r (1 cy/row on PE vs 4 for float32;
          ~12-bit mantissa operand rounding, fp32 accumulate).
  "qk32": q/k projection + scores matmul in float32 (full precision on the
          exp-amplified path), everything else float32r.
  "fp32": everything float32.
"""

import numpy as np
from contextlib import ExitStack

import concourse.mybir as mybir
import concourse.tile as tile
from concourse import bacc
from concourse.bass_utils import run_bass_kernel_spmd
from concourse.masks import make_identity

F32 = mybir.dt.float32
F32R = mybir.dt.float32r
AF = mybir.ActivationFunctionType

C = 1024
H = 16
D = 64
N_CORES = 8
HPC = H // N_CORES          # heads per core = 2
ROPE_BASE = 10000.0
KC = C // 128               # contraction chunks for the qkv projection = 8

RMODE = "all"


def build_program(B, T, use_qk_bias, use_v_bias, rmode=RMODE, n_cores=N_CORES):
    TOK = B * T
    NB = TOK // 512           # 512-token blocks
    NCHUNK = TOK // 128       # 128-token chunks (v storage)
    QB = T // 512             # q-blocks per sequence
    CS = T // 512             # distinct 512-col blocks of the rope tables

    # rmode: "all" | "qk32" | "fp32", or a 3-tuple of dtypes
    # (d_qkproj, d_attn, d_out) for bisection.
    if isinstance(rmode, tuple):
        QKD, AD, OD = rmode
    elif rmode == "all":
        QKD = AD = OD = F32R
    elif rmode == "qk32":
        QKD, AD, OD = F32, F32R, F32R
    else:
        QKD = AD = OD = F32

    nc = bacc.Bacc("TRN2", target_bir_lowering=False, debug=False,
                   num_devices=n_cores)

    xT = nc.dram_tensor("xT", [C, TOK], QKD, kind="ExternalInput").ap()
    wq = nc.dram_tensor("wq", [C, 128], QKD, kind="ExternalInput").ap()
    wk = nc.dram_tensor("wk", [C, 128], QKD, kind="ExternalInput").ap()
    wv = nc.dram_tensor("wv", [C, 128], AD, kind="ExternalInput").ap()
    wo = nc.dram_tensor("wo", [128, C], OD, kind="ExternalInput").ap()
    cosT = nc.dram_tensor("cosT", [128, T], F32, kind="ExternalInput").ap()
    sinT = nc.dram_tensor("sinT", [128, T], F32, kind="ExternalInput").ap()
    msk = nc.dram_tensor("msk", [128, 2048], F32, kind="ExternalInput").ap()
    rot = nc.dram_tensor("rot", [128, 128], AD, kind="ExternalInput").ap()
    if use_qk_bias:
        bq = nc.dram_tensor("bq", [128, 1], F32, kind="ExternalInput").ap()
        bk = nc.dram_tensor("bk", [128, 1], F32, kind="ExternalInput").ap()
    if use_v_bias:
        bv = nc.dram_tensor("bv", [128, 1], F32, kind="ExternalInput").ap()
    outp = nc.dram_tensor("outp", [TOK, C], F32, kind="ExternalOutput").ap()

    with tile.TileContext(nc) as tc:
        with ExitStack() as res:  # tensors resident through phases 1+2
            persist = res.enter_context(tc.tile_pool(name="persist", bufs=1))
            qT = persist.tile([128, TOK], AD)
            kT = persist.tile([128, TOK], AD)
            vsb = persist.tile([128, NCHUNK * 130], AD)
            ones_sb = persist.tile([1, 64], AD)
            nc.vector.memset(ones_sb[:].bitcast(F32) if AD == F32R else ones_sb[:], 1.0)

            with tc.tile_pool(name="ot", bufs=1) as otpool:
                OT = otpool.tile([128, TOK], OD)

                # ---------------- phase 1: qkv projection + RoPE ----------
                with ExitStack() as p1:
                    cpool = p1.enter_context(tc.tile_pool(name="p1c", bufs=1))
                    wq_sb = cpool.tile([128, C], QKD)
                    wk_sb = cpool.tile([128, C], QKD)
                    wv_sb = cpool.tile([128, C], AD)
                    cos_sb = cpool.tile([128, T], F32)
                    sin_sb = cpool.tile([128, T], F32)
                    rot_sb = cpool.tile([128, 128], AD)
                    ident = cpool.tile([128, 128], F32)
                    make_identity(nc, ident[:])
                    for k in range(KC):
                        nc.sync.dma_start(wq_sb[:, k * 128:(k + 1) * 128],
                                          wq[k * 128:(k + 1) * 128, :])
                        nc.sync.dma_start(wk_sb[:, k * 128:(k + 1) * 128],
                                          wk[k * 128:(k + 1) * 128, :])
                        nc.sync.dma_start(wv_sb[:, k * 128:(k + 1) * 128],
                                          wv[k * 128:(k + 1) * 128, :])
                    nc.sync.dma_start(cos_sb[:], cosT[:])
                    nc.sync.dma_start(sin_sb[:], sinT[:])
                    nc.sync.dma_start(rot_sb[:], rot[:])
                    if use_qk_bias:
                        bq_sb = cpool.tile([128, 1], F32)
                        bk_sb = cpool.tile([128, 1], F32)
                        nc.sync.dma_start(bq_sb[:], bq[:])
                        nc.sync.dma_start(bk_sb[:], bk[:])
                    if use_v_bias:
                        bv_sb = cpool.tile([128, 1], F32)
                        nc.sync.dma_start(bv_sb[:], bv[:])

                    xpool = p1.enter_context(tc.tile_pool(name="xp", bufs=8))
                    xrpool = (xpool if QKD == AD else
                              p1.enter_context(tc.tile_pool(name="xrp", bufs=6)))
                    qkpsum = p1.enter_context(
                        tc.tile_pool(name="qkp", bufs=2, space="PSUM"))
                    rotpsum = p1.enter_context(
                        tc.tile_pool(name="rotp", bufs=1, space="PSUM"))
                    vpsum = p1.enter_context(
                        tc.tile_pool(name="vp", bufs=2, space="PSUM"))
                    tppsum = p1.enter_context(
                        tc.tile_pool(name="tpp", bufs=2, space="PSUM"))
                    tmp = p1.enter_context(tc.tile_pool(name="tmp", bufs=6))

                    for nb in range(NB):
                        t512 = slice(nb * 512, (nb + 1) * 512)
                        cs = slice((nb % CS) * 512, (nb % CS) * 512 + 512)
                        xc = []
                        for k in range(KC):
                            t = xpool.tile([128, 512], QKD, tag="xc")
                            nc.sync.dma_start(
                                t[:], xT[k * 128:(k + 1) * 128, t512])
                            xc.append(t)
                        if QKD == AD:
                            xcv = xc
                        else:
                            # round a second copy for the fp32r v projection
                            xcv = []
                            for k in range(KC):
                                t = xrpool.tile([128, 512], AD, tag="xcv")
                                nc.gpsimd.tensor_copy(t[:], xc[k][:])
                                xcv.append(t)
                        for w_sb, b_name, dstT in ((wq_sb, "bq", qT),
                                                   (wk_sb, "bk", kT)):
                            acc = qkpsum.tile([128, 512], F32, tag="acc")
                            for k in range(KC):
                                nc.tensor.matmul(
                                    acc[:], w_sb[:, k * 128:(k + 1) * 128],
                                    xc[k][:], start=(k == 0), stop=(k == KC - 1))
                            raw = tmp.tile([128, 512], AD, tag="ropetmp")
                            if use_qk_bias:
                                b_sb = bq_sb if b_name == "bq" else bk_sb
                                nc.vector.tensor_scalar_add(raw[:], acc[:],
                                                            b_sb[:])
                            else:
                                nc.vector.tensor_copy(raw[:], acc[:])
                            rp = rotpsum.tile([128, 512], F32, tag="rp")
                            nc.tensor.matmul(rp[:], rot_sb[:], raw[:],
                                             start=True, stop=True)
                            t1 = tmp.tile([128, 512], F32, tag="ropetmp")
                            nc.vector.tensor_mul(t1[:], raw[:], cos_sb[:, cs])
                            t2 = tmp.tile([128, 512], F32, tag="ropetmp")
                            nc.vector.tensor_mul(t2[:], rp[:], sin_sb[:, cs])
                            nc.vector.tensor_add(dstT[:, t512], t1[:], t2[:])
                        # v: project transposed (N=512 streams), then PE-
                        # transpose back to [tokens, 128] chunks
                        vacc = vpsum.tile([128, 512], F32, tag="vacc")
                        for k in range(KC):
                            nc.tensor.matmul(
                                vacc[:], wv_sb[:, k * 128:(k + 1) * 128],
                                xcv[k][:], start=(k == 0), stop=(k == KC - 1))
                        vraw = tmp.tile([128, 512], F32, tag="vraw", bufs=2)
                        if use_v_bias:
                            nc.vector.tensor_scalar_add(vraw[:], vacc[:],
                                                        bv_sb[:])
                        else:
                            nc.vector.tensor_copy(vraw[:], vacc[:])
                        for sub in range(4):
                            tp = tppsum.tile([128, 128], F32, tag="tp")
                            nc.tensor.transpose(
                                tp[:], vraw[:, sub * 128:(sub + 1) * 128],
                                ident[:])
                            base = (nb * 4 + sub) * 130
                            nc.vector.tensor_copy(vsb[:, base:base + 64],
                                                  tp[:, 0:64])
                            nc.vector.tensor_copy(vsb[:, base + 65:base + 129],
                                                  tp[:, 64:128])
                            nc.vector.memset(vsb[:, base + 64:base + 65].bitcast(F32) if AD == F32R else vsb[:, base + 64:base + 65], 1.0)
                            nc.vector.memset(vsb[:, base + 129:base + 130].bitcast(F32) if AD == F32R else vsb[:, base + 129:base + 130], 1.0)

                # ---------------- phase 2: causal attention ---------------
                with ExitStack() as p2:
                    mpool = p2.enter_context(tc.tile_pool(name="mp", bufs=1))
                    msk_sb = mpool.tile([128, 2048], F32)
                    nc.sync.dma_start(msk_sb[:], msk[:])
                    spsum = p2.enter_context(
                        tc.tile_pool(name="sp", bufs=3, space="PSUM"))
                    popsum = p2.enter_context(
                        tc.tile_pool(name="pop", bufs=2, space="PSUM"))
                    bcpsum = p2.enter_context(
                        tc.tile_pool(name="bcp", bufs=2, space="PSUM"))
                    ppool = p2.enter_context(tc.tile_pool(name="pp", bufs=3))
                    dpool = p2.enter_context(tc.tile_pool(name="dp", bufs=2))
                    bspool = p2.enter_context(tc.tile_pool(name="bs", bufs=2))

                    for b in range(B):
                        t0 = b * T
                        c0 = b * (T // 128)
                        for hp in range(HPC):
                            hs = slice(hp * 64, (hp + 1) * 64)
                            for qb in range(QB):
                                q512 = slice(t0 + qb * 512, t0 + (qb + 1) * 512)
                                po = popsum.tile([65, 512], F32, tag="po")
                                ns = (qb + 1) * 4
                                for si in range(ns):
                                    S = spsum.tile([128, 512], F32, tag="S")
                                    s0 = t0 + si * 128
                                    nc.tensor.matmul(
                                        S[:], kT[hs, s0:s0 + 128],
                                        qT[hs, q512], start=True, stop=True)
                                    P = ppool.tile([128, 512], AD, tag="P")
                                    nc.scalar.activation(P[:], S[:], AF.Exp)
                                    off = si * 128 - qb * 512
                                    if off >= 0:
                                        vi = off // 128
                                        nc.vector.tensor_mul(
                                            P[:], P[:],
                                            msk_sb[:, vi * 512:(vi + 1) * 512])
                                    vbase = (c0 + si) * 130 + hp * 65
                                    nc.tensor.matmul(
                                        po[:], vsb[:, vbase:vbase + 65], P[:],
                                        start=(si == 0), stop=(si == ns - 1))
                                den32 = dpool.tile([1, 512], F32, tag="den32")
                                nc.vector.reciprocal(den32[:], po[64:65, :])
                                den = dpool.tile([1, 512], AD, tag="den")
                                nc.vector.tensor_copy(den[:], den32[:])
                                bc = bcpsum.tile([64, 512], F32, tag="bc")
                                nc.tensor.matmul(bc[:], ones_sb[:], den[:],
                                                 start=True, stop=True)
                                bcs = bspool.tile([64, 512], F32, tag="bcs")
                                nc.vector.tensor_copy(bcs[:], bc[:])
                                nc.vector.tensor_mul(
                                    OT[hs, q512], po[0:64, :], bcs[:])

                # ---------------- phase 3: output projection --------------
                with ExitStack() as p3:
                    wpool = p3.enter_context(tc.tile_pool(name="wop", bufs=1))
                    wo_sb = wpool.tile([128, C], OD)
                    nc.sync.dma_start(wo_sb[:], wo[:])
                    opsum = p3.enter_context(
                        tc.tile_pool(name="op", bufs=4, space="PSUM"))
                    ostage = p3.enter_context(tc.tile_pool(name="os", bufs=4))
                    for ci in range(NCHUNK):
                        for n2 in range(C // 512):
                            pacc = opsum.tile([128, 512], F32, tag="pacc")
                            nc.tensor.matmul(
                                pacc[:], OT[:, ci * 128:(ci + 1) * 128],
                                wo_sb[:, n2 * 512:(n2 + 1) * 512],
                                start=True, stop=True)
                            osb = ostage.tile([128, 512], F32, tag="osb")
                            if (ci + n2) % 2 == 0:
                                nc.scalar.activation(osb[:], pacc[:], AF.Copy)
                            else:
                                nc.vector.tensor_copy(osb[:], pacc[:])
                            nc.sync.dma_start(
                                outp[ci * 128:(ci + 1) * 128,
                                     n2 * 512:(n2 + 1) * 512], osb[:])

    nc.compile()
    return nc


def make_rope_tables(T, dtype=np.float32):
    j = np.arange(32, dtype=np.float32)
    inv_freq = (1.0 / (ROPE_BASE ** (2.0 * j / D))).astype(np.float32)
    t = np.arange(T, dtype=np.float32)
    freqs = t[None, :] * inv_freq[:, None]          # [32, T]
    half = np.concatenate([freqs, freqs], axis=0)   # [64, T]
    cosT = np.cos(half).astype(dtype)
    sinT = np.sin(half).astype(dtype)
    return (np.concatenate([cosT, cosT], axis=0),   # [128, T] (2 heads)
            np.concatenate([sinT, sinT], axis=0))


def make_rot_matrix():
    """lhsT R [128,128] s.t. (R.T @ x)[m] = rotate_half(x)[m] per 64-row head."""
    R = np.zeros((128, 128), dtype=np.float32)
    for hb in (0, 64):
        for m in range(32):
            R[hb + m + 32, hb + m] = -1.0
            R[hb + m, hb + m + 32] = 1.0
    return R


def make_masks():
    """[128, 4*512]: variant vi valid where s + vi*128 <= q."""
    s = np.arange(128)[:, None]
    q = np.arange(512)[None, :]
    blocks = [(s + vi * 128 <= q).astype(np.float32) for vi in range(4)]
    return np.concatenate(blocks, axis=1)


def prep_in_maps(x, W_qkv, b_qkv, W_out, B, T, use_qk_bias, use_v_bias,
                 n_cores=N_CORES):
    TOK = B * T
    xTm = np.ascontiguousarray(x.reshape(TOK, C).T)
    cosT, sinT = make_rope_tables(T)
    msk = make_masks()
    rot = make_rot_matrix()
    in_maps = []
    for c in range(n_cores):
        h0 = c * HPC
        cols = slice(h0 * D, (h0 + HPC) * D)        # 128 head-dim columns
        m = {
            "xT": xTm,
            "wq": np.ascontiguousarray(W_qkv[:, cols]),
            "wk": np.ascontiguousarray(W_qkv[:, C:][:, cols]),
            "wv": np.ascontiguousarray(W_qkv[:, 2 * C:][:, cols]),
            "wo": np.ascontiguousarray(W_out[cols, :]),
            "cosT": cosT, "sinT": sinT, "msk": msk, "rot": rot,
        }
        if use_qk_bias:
            m["bq"] = np.ascontiguousarray(b_qkv[cols]).reshape(128, 1)
            m["bk"] = np.ascontiguousarray(b_qkv[C:][cols]).reshape(128, 1)
        if use_v_bias:
            m["bv"] = np.ascontiguousarray(b_qkv[2 * C:][cols]).reshape(128, 1)
        in_maps.append(m)
    return in_maps


_CACHE = {}


def _get_program(key):
    if key not in _CACHE:
        B, T, use_qk_bias, use_v_bias = key
        _CACHE[key] = build_program(B, T, use_qk_bias, use_v_bias)
    return _CACHE[key]


def kernel(x, W_qkv, b_qkv, W_out, b_out):
    x = np.asarray(x, dtype=np.float32)
    W_qkv = np.asarray(W_qkv, dtype=np.float32)
    b_qkv = np.asarray(b_qkv, dtype=np.float32)
    W_out = np.asarray(W_out, dtype=np.float32)
    b_out = np.asarray(b_out, dtype=np.float32)
    B, T, _ = x.shape
    use_qk_bias = bool(np.any(b_qkv[:2 * C]))
    use_v_bias = bool(np.any(b_qkv[2 * C:]))
    nc = _get_program((B, T, use_qk_bias, use_v_bias))
    in_maps = prep_in_maps(x, W_qkv, b_qkv, W_out, B, T,
                           use_qk_bias, use_v_bias)
    res = run_bass_kernel_spmd(nc, in_maps, list(range(N_CORES)))
    acc = res.results[0]["outp"].astype(np.float32)
    for c in range(1, N_CORES):
        acc = acc + res.results[c]["outp"]
    acc = acc + b_out[None, :]
    return acc.reshape(B, T, C)


# revision 8
# speedup vs baseline: 2.0367x; 1.2137x over previous
"""Trainium2 Bass kernel for causal self-attention with RoPE.

Problem: y = CausalSelfAttention(x) with
  B, T, C, H = 4, 2048, 1024, 16; D = 64; RoPE base 10000; no 1/sqrt(D) scale.

Sharding: Megatron-style tensor parallel over heads. 8 cores, 2 heads each.
Each core computes qkv for its 2 heads (columns of W_qkv), runs attention for
its (b, head) pairs, and multiplies by its 128 rows of W_out, producing a
partial (B*T, C) output. The host sums the 8 partials and adds b_out.

Per-core device pipeline:
  phase 1: qT/kT/vT [128, B*T] (2 heads stacked on partitions) via PE matmuls
           with W chunks stationary; RoPE on q/k via a +-1 permutation matmul
           (rotate_half) + DVE mul/adds; v transposed back to [tokens, 128]
           via PE transpose, stored chunked with an interleaved ones column.
  phase 2: per (b, head): S^T = kT_chunk^T @ qT_block on PE (s on partitions),
           exp on ACT (no max subtraction: |score| <~ 60, fp32 exp can't
           overflow), causal mask via 0/1 multiply, P^T @ [v | 1] accumulated
           on PE -> O^T rows + denominator row, normalize via reciprocal +
           K=1 broadcast matmul.
  phase 3: out_partial[tokens, C] = O^T.T @ W_out_rows on PE.

Matmul dtype strategy (RMODE):
  "all":  every matmul in float32r (1 cy/row on PE vs 4 for float32;
          ~12-bit mantissa operand rounding, fp32 accumulate).
  "qk32": q/k projection + scores matmul in float32 (full precision on the
          exp-amplified path), everything else float32r.
  "fp32": everything float32.
"""

import numpy as np
from contextlib import ExitStack

import concourse.mybir as mybir
import concourse.tile as tile
from concourse import bacc
from concourse.bass_utils import run_bass_kernel_spmd
from concourse.masks import make_identity

F32 = mybir.dt.float32
F32R = mybir.dt.float32r
AF = mybir.ActivationFunctionType

C = 1024
H = 16
D = 64
N_CORES = 8
HPC = H // N_CORES          # heads per core = 2
ROPE_BASE = 10000.0
KC = C // 128               # contraction chunks for the qkv projection = 8

RMODE = "qk32"


def build_program(B, T, use_qk_bias, use_v_bias, rmode=RMODE, n_cores=N_CORES):
    TOK = B * T
    NB = TOK // 512           # 512-token blocks
    NCHUNK = TOK // 128       # 128-token chunks (v storage)
    QB = T // 512             # q-blocks per sequence
    CS = T // 512             # distinct 512-col blocks of the rope tables

    # rmode: "all" | "qk32" | "fp32", or a 3-tuple of dtypes
    # (d_qkproj, d_attn, d_out) for bisection.
    if isinstance(rmode, tuple):
        QKD, AD, OD = rmode
    elif rmode == "all":
        QKD = AD = OD = F32R
    elif rmode == "qk32":
        QKD, AD, OD = F32, F32R, F32R
    else:
        QKD = AD = OD = F32

    nc = bacc.Bacc("TRN2", target_bir_lowering=False, debug=False,
                   num_devices=n_cores)

    xT = nc.dram_tensor("xT", [C, TOK], QKD, kind="ExternalInput").ap()
    wq = nc.dram_tensor("wq", [C, 128], QKD, kind="ExternalInput").ap()
    wk = nc.dram_tensor("wk", [C, 128], QKD, kind="ExternalInput").ap()
    wv = nc.dram_tensor("wv", [C, 128], AD, kind="ExternalInput").ap()
    wo = nc.dram_tensor("wo", [128, C], OD, kind="ExternalInput").ap()
    cosT = nc.dram_tensor("cosT", [128, T], F32, kind="ExternalInput").ap()
    sinT = nc.dram_tensor("sinT", [128, T], F32, kind="ExternalInput").ap()
    msk = nc.dram_tensor("msk", [128, 2048], F32, kind="ExternalInput").ap()
    rot = nc.dram_tensor("rot", [128, 128], AD, kind="ExternalInput").ap()
    if use_qk_bias:
        bq = nc.dram_tensor("bq", [128, 1], F32, kind="ExternalInput").ap()
        bk = nc.dram_tensor("bk", [128, 1], F32, kind="ExternalInput").ap()
    if use_v_bias:
        bv = nc.dram_tensor("bv", [128, 1], F32, kind="ExternalInput").ap()
    outp = nc.dram_tensor("outp", [TOK, C], F32, kind="ExternalOutput").ap()

    with tile.TileContext(nc) as tc:
        with ExitStack() as res:  # tensors resident through phases 1+2
            persist = res.enter_context(tc.tile_pool(name="persist", bufs=1))
            qT = persist.tile([128, TOK], AD)
            kT = persist.tile([128, TOK], AD)
            vsb = persist.tile([128, NCHUNK * 130], AD)
            ones_sb = persist.tile([1, 64], AD)
            nc.vector.memset(ones_sb[:].bitcast(F32) if AD == F32R else ones_sb[:], 1.0)

            with tc.tile_pool(name="ot", bufs=1) as otpool:
                OT = otpool.tile([128, TOK], OD)

                # ---------------- phase 1: qkv projection + RoPE ----------
                with ExitStack() as p1:
                    cpool = p1.enter_context(tc.tile_pool(name="p1c", bufs=1))
                    wq_sb = cpool.tile([128, C], QKD)
                    wk_sb = cpool.tile([128, C], QKD)
                    wv_sb = cpool.tile([128, C], AD)
                    cos_sb = cpool.tile([128, T], F32)
                    sin_sb = cpool.tile([128, T], F32)
                    rot_sb = cpool.tile([128, 128], AD)
                    ident = cpool.tile([128, 128], F32)
                    make_identity(nc, ident[:])
                    for k in range(KC):
                        nc.sync.dma_start(wq_sb[:, k * 128:(k + 1) * 128],
                                          wq[k * 128:(k + 1) * 128, :])
                        nc.sync.dma_start(wk_sb[:, k * 128:(k + 1) * 128],
                                          wk[k * 128:(k + 1) * 128, :])
                        nc.sync.dma_start(wv_sb[:, k * 128:(k + 1) * 128],
                                          wv[k * 128:(k + 1) * 128, :])
                    nc.sync.dma_start(cos_sb[:], cosT[:])
                    nc.sync.dma_start(sin_sb[:], sinT[:])
                    nc.sync.dma_start(rot_sb[:], rot[:])
                    if use_qk_bias:
                        bq_sb = cpool.tile([128, 1], F32)
                        bk_sb = cpool.tile([128, 1], F32)
                        nc.sync.dma_start(bq_sb[:], bq[:])
                        nc.sync.dma_start(bk_sb[:], bk[:])
                    if use_v_bias:
                        bv_sb = cpool.tile([128, 1], F32)
                        nc.sync.dma_start(bv_sb[:], bv[:])

                    xpool = p1.enter_context(tc.tile_pool(name="xp", bufs=8))
                    xrpool = (xpool if QKD == AD else
                              p1.enter_context(tc.tile_pool(name="xrp", bufs=6)))
                    qkpsum = p1.enter_context(
                        tc.tile_pool(name="qkp", bufs=2, space="PSUM"))
                    rotpsum = p1.enter_context(
                        tc.tile_pool(name="rotp", bufs=1, space="PSUM"))
                    vpsum = p1.enter_context(
                        tc.tile_pool(name="vp", bufs=2, space="PSUM"))
                    tppsum = p1.enter_context(
                        tc.tile_pool(name="tpp", bufs=2, space="PSUM"))
                    tmp = p1.enter_context(tc.tile_pool(name="tmp", bufs=6))

                    for nb in range(NB):
                        t512 = slice(nb * 512, (nb + 1) * 512)
                        cs = slice((nb % CS) * 512, (nb % CS) * 512 + 512)
                        xc = []
                        for k in range(KC):
                            t = xpool.tile([128, 512], QKD, tag="xc")
                            nc.sync.dma_start(
                                t[:], xT[k * 128:(k + 1) * 128, t512])
                            xc.append(t)
                        if QKD == AD:
                            xcv = xc
                        else:
                            # round a second copy for the fp32r v projection
                            xcv = []
                            for k in range(KC):
                                t = xrpool.tile([128, 512], AD, tag="xcv")
                                nc.gpsimd.tensor_copy(t[:], xc[k][:])
                                xcv.append(t)
                        for w_sb, b_name, dstT in ((wq_sb, "bq", qT),
                                                   (wk_sb, "bk", kT)):
                            acc = qkpsum.tile([128, 512], F32, tag="acc")
                            for k in range(KC):
                                nc.tensor.matmul(
                                    acc[:], w_sb[:, k * 128:(k + 1) * 128],
                                    xc[k][:], start=(k == 0), stop=(k == KC - 1))
                            raw = tmp.tile([128, 512], AD, tag="ropetmp")
                            if use_qk_bias:
                                b_sb = bq_sb if b_name == "bq" else bk_sb
                                nc.vector.tensor_scalar_add(raw[:], acc[:],
                                                            b_sb[:])
                            else:
                                nc.vector.tensor_copy(raw[:], acc[:])
                            rp = rotpsum.tile([128, 512], F32, tag="rp")
                            nc.tensor.matmul(rp[:], rot_sb[:], raw[:],
                                             start=True, stop=True)
                            t1 = tmp.tile([128, 512], F32, tag="ropetmp")
                            nc.vector.tensor_mul(t1[:], raw[:], cos_sb[:, cs])
                            t2 = tmp.tile([128, 512], F32, tag="ropetmp")
                            nc.vector.tensor_mul(t2[:], rp[:], sin_sb[:, cs])
                            nc.vector.tensor_add(dstT[:, t512], t1[:], t2[:])
                        # v: project transposed (N=512 streams), then PE-
                        # transpose back to [tokens, 128] chunks
                        vacc = vpsum.tile([128, 512], F32, tag="vacc")
                        for k in range(KC):
                            nc.tensor.matmul(
                                vacc[:], wv_sb[:, k * 128:(k + 1) * 128],
                                xcv[k][:], start=(k == 0), stop=(k == KC - 1))
                        vraw = tmp.tile([128, 512], F32, tag="vraw", bufs=2)
                        if use_v_bias:
                            nc.vector.tensor_scalar_add(vraw[:], vacc[:],
                                                        bv_sb[:])
                        else:
                            nc.vector.tensor_copy(vraw[:], vacc[:])
                        for sub in range(4):
                            tp = tppsum.tile([128, 128], F32, tag="tp")
                            nc.tensor.transpose(
                                tp[:], vraw[:, sub * 128:(sub + 1) * 128],
                                ident[:])
                            base = (nb * 4 + sub) * 130
                            nc.vector.tensor_copy(vsb[:, base:base + 64],
                                                  tp[:, 0:64])
                            nc.vector.tensor_copy(vsb[:, base + 65:base + 129],
                                                  tp[:, 64:128])
                            nc.vector.memset(vsb[:, base + 64:base + 65].bitcast(F32) if AD == F32R else vsb[:, base + 64:base + 65], 1.0)
                            nc.vector.memset(vsb[:, base + 129:base + 130].bitcast(F32) if AD == F32R else vsb[:, base + 129:base + 130], 1.0)

                # ---------------- phase 2: causal attention ---------------
                with ExitStack() as p2:
                    mpool = p2.enter_context(tc.tile_pool(name="mp", bufs=1))
                    msk_sb = mpool.tile([128, 2048], F32)
                    nc.sync.dma_start(msk_sb[:], msk[:])
                    spsum = p2.enter_context(
                        tc.tile_pool(name="sp", bufs=3, space="PSUM"))
                    popsum = p2.enter_context(
                        tc.tile_pool(name="pop", bufs=2, space="PSUM"))
                    bcpsum = p2.enter_context(
                        tc.tile_pool(name="bcp", bufs=2, space="PSUM"))
                    ppool = p2.enter_context(tc.tile_pool(name="pp", bufs=3))
                    dpool = p2.enter_context(tc.tile_pool(name="dp", bufs=2))
                    bspool = p2.enter_context(tc.tile_pool(name="bs", bufs=2))

                    for b in range(B):
                        t0 = b * T
                        c0 = b * (T // 128)
                        for hp in range(HPC):
                            hs = slice(hp * 64, (hp + 1) * 64)
                            for qb in range(QB):
                                q512 = slice(t0 + qb * 512, t0 + (qb + 1) * 512)
                                po = popsum.tile([65, 512], F32, tag="po")
                                ns = (qb + 1) * 4
                                for si in range(ns):
                                    S = spsum.tile([128, 512], F32, tag="S")
                                    s0 = t0 + si * 128
                                    nc.tensor.matmul(
                                        S[:], kT[hs, s0:s0 + 128],
                                        qT[hs, q512], start=True, stop=True)
                                    P = ppool.tile([128, 512], AD, tag="P")
                                    nc.scalar.activation(P[:], S[:], AF.Exp)
                                    off = si * 128 - qb * 512
                                    if off >= 0:
                                        vi = off // 128
                                        nc.vector.tensor_mul(
                                            P[:], P[:],
                                            msk_sb[:, vi * 512:(vi + 1) * 512])
                                    vbase = (c0 + si) * 130 + hp * 65
                                    nc.tensor.matmul(
                                        po[:], vsb[:, vbase:vbase + 65], P[:],
                                        start=(si == 0), stop=(si == ns - 1))
                                den32 = dpool.tile([1, 512], F32, tag="den32")
                                nc.vector.reciprocal(den32[:], po[64:65, :])
                                den = dpool.tile([1, 512], AD, tag="den")
                                nc.vector.tensor_copy(den[:], den32[:])
                                bc = bcpsum.tile([64, 512], F32, tag="bc")
                                nc.tensor.matmul(bc[:], ones_sb[:], den[:],
                                                 start=True, stop=True)
                                bcs = bspool.tile([64, 512], F32, tag="bcs")
                                nc.vector.tensor_copy(bcs[:], bc[:])
                                nc.vector.tensor_mul(
                                    OT[hs, q512], po[0:64, :], bcs[:])

                # ---------------- phase 3: output projection --------------
                with ExitStack() as p3:
                    wpool = p3.enter_context(tc.tile_pool(name="wop", bufs=1))
                    wo_sb = wpool.tile([128, C], OD)
                    nc.sync.dma_start(wo_sb[:], wo[:])
                    opsum = p3.enter_context(
                        tc.tile_pool(name="op", bufs=4, space="PSUM"))
                    ostage = p3.enter_context(tc.tile_pool(name="os", bufs=4))
                    for ci in range(NCHUNK):
                        for n2 in range(C // 512):
                            pacc = opsum.tile([128, 512], F32, tag="pacc")
                            nc.tensor.matmul(
                                pacc[:], OT[:, ci * 128:(ci + 1) * 128],
                                wo_sb[:, n2 * 512:(n2 + 1) * 512],
                                start=True, stop=True)
                            osb = ostage.tile([128, 512], F32, tag="osb")
                            if (ci + n2) % 2 == 0:
                                nc.scalar.activation(osb[:], pacc[:], AF.Copy)
                            else:
                                nc.vector.tensor_copy(osb[:], pacc[:])
                            nc.sync.dma_start(
                                outp[ci * 128:(ci + 1) * 128,
                                     n2 * 512:(n2 + 1) * 512], osb[:])

    nc.compile()
    return nc


def make_rope_tables(T, dtype=np.float32):
    j = np.arange(32, dtype=np.float32)
    inv_freq = (1.0 / (ROPE_BASE ** (2.0 * j / D))).astype(np.float32)
    t = np.arange(T, dtype=np.float32)
    freqs = t[None, :] * inv_freq[:, None]          # [32, T]
    half = np.concatenate([freqs, freqs], axis=0)   # [64, T]
    cosT = np.cos(half).astype(dtype)
    sinT = np.sin(half).astype(dtype)
    return (np.concatenate([cosT, cosT], axis=0),   # [128, T] (2 heads)
            np.concatenate([sinT, sinT], axis=0))


def make_rot_matrix():
    """lhsT R [128,128] s.t. (R.T @ x)[m] = rotate_half(x)[m] per 64-row head."""
    R = np.zeros((128, 128), dtype=np.float32)
    for hb in (0, 64):
        for m in range(32):
            R[hb + m + 32, hb + m] = -1.0
            R[hb + m, hb + m + 32] = 1.0
    return R


def make_masks():
    """[128, 4*512]: variant vi valid where s + vi*128 <= q."""
    s = np.arange(128)[:, None]
    q = np.arange(512)[None, :]
    blocks = [(s + vi * 128 <= q).astype(np.float32) for vi in range(4)]
    return np.concatenate(blocks, axis=1)


def prep_in_maps(x, W_qkv, b_qkv, W_out, B, T, use_qk_bias, use_v_bias,
                 n_cores=N_CORES):
    TOK = B * T
    xTm = np.ascontiguousarray(x.reshape(TOK, C).T)
    cosT, sinT = make_rope_tables(T)
    msk = make_masks()
    rot = make_rot_matrix()
    in_maps = []
    for c in range(n_cores):
        h0 = c * HPC
        cols = slice(h0 * D, (h0 + HPC) * D)        # 128 head-dim columns
        m = {
            "xT": xTm,
            "wq": np.ascontiguousarray(W_qkv[:, cols]),
            "wk": np.ascontiguousarray(W_qkv[:, C:][:, cols]),
            "wv": np.ascontiguousarray(W_qkv[:, 2 * C:][:, cols]),
            "wo": np.ascontiguousarray(W_out[cols, :]),
            "cosT": cosT, "sinT": sinT, "msk": msk, "rot": rot,
        }
        if use_qk_bias:
            m["bq"] = np.ascontiguousarray(b_qkv[cols]).reshape(128, 1)
            m["bk"] = np.ascontiguousarray(b_qkv[C:][cols]).reshape(128, 1)
        if use_v_bias:
            m["bv"] = np.ascontiguousarray(b_qkv[2 * C:][cols]).reshape(128, 1)
        in_maps.append(m)
    return in_maps


_CACHE = {}


def _get_program(key):
    if key not in _CACHE:
        B, T, use_qk_bias, use_v_bias = key
        _CACHE[key] = build_program(B, T, use_qk_bias, use_v_bias)
    return _CACHE[key]


def kernel(x, W_qkv, b_qkv, W_out, b_out):
    x = np.asarray(x, dtype=np.float32)
    W_qkv = np.asarray(W_qkv, dtype=np.float32)
    b_qkv = np.asarray(b_qkv, dtype=np.float32)
    W_out = np.asarray(W_out, dtype=np.float32)
    b_out = np.asarray(b_out, dtype=np.float32)
    B, T, _ = x.shape
    use_qk_bias = bool(np.any(b_qkv[:2 * C]))
    use_v_bias = bool(np.any(b_qkv[2 * C:]))
    nc = _get_program((B, T, use_qk_bias, use_v_bias))
    in_maps = prep_in_maps(x, W_qkv, b_qkv, W_out, B, T,
                           use_qk_bias, use_v_bias)
    res = run_bass_kernel_spmd(nc, in_maps, list(range(N_CORES)))
    acc = res.results[0]["outp"].astype(np.float32)
    for c in range(1, N_CORES):
        acc = acc + res.results[c]["outp"]
    acc = acc + b_out[None, :]
    return acc.reshape(B, T, C)


# revision 9
# speedup vs baseline: 2.0644x; 1.0136x over previous
"""Trainium2 Bass kernel for causal self-attention with RoPE.

Problem: y = CausalSelfAttention(x) with
  B, T, C, H = 4, 2048, 1024, 16; D = 64; RoPE base 10000; no 1/sqrt(D) scale.

Sharding: Megatron-style tensor parallel over heads. 8 cores, 2 heads each.
Each core computes qkv for its 2 heads (columns of W_qkv), runs attention for
its (b, head) pairs, and multiplies by its 128 rows of W_out, producing a
partial (B*T, C) output. The host sums the 8 partials and adds b_out.

Per-core device pipeline:
  phase 1: qT/kT/vT [128, B*T] (2 heads stacked on partitions) via PE matmuls
           with W chunks stationary; RoPE on q/k via a +-1 permutation matmul
           (rotate_half) + DVE mul/adds; v transposed back to [tokens, 128]
           via PE transpose, stored chunked with an interleaved ones column.
  phase 2: per (b, head): S^T = kT_chunk^T @ qT_block on PE (s on partitions),
           exp on ACT (no max subtraction: |score| <~ 60, fp32 exp can't
           overflow), causal mask via 0/1 multiply, P^T @ [v | 1] accumulated
           on PE -> O^T rows + denominator row, normalize via reciprocal +
           K=1 broadcast matmul.
  phase 3: out_partial[tokens, C] = O^T.T @ W_out_rows on PE.

Matmul dtype strategy (RMODE):
  "all":  every matmul in float32r (1 cy/row on PE vs 4 for float32;
          ~12-bit mantissa operand rounding, fp32 accumulate).
  "qk32": q/k projection + scores matmul in float32 (full precision on the
          exp-amplified path), everything else float32r.
  "fp32": everything float32.
"""

import numpy as np
from contextlib import ExitStack

import concourse.mybir as mybir
import concourse.tile as tile
from concourse import bacc
from concourse.bass_utils import run_bass_kernel_spmd
from concourse.masks import make_identity

F32 = mybir.dt.float32
F32R = mybir.dt.float32r
AF = mybir.ActivationFunctionType

C = 1024
H = 16
D = 64
N_CORES = 8
HPC = H // N_CORES          # heads per core = 2
ROPE_BASE = 10000.0
KC = C // 128               # contraction chunks for the qkv projection = 8

RMODE = "all"


def build_program(B, T, use_qk_bias, use_v_bias, rmode=RMODE, n_cores=N_CORES):
    TOK = B * T
    NB = TOK // 512           # 512-token blocks
    NCHUNK = TOK // 128       # 128-token chunks (v storage)
    QB = T // 512             # q-blocks per sequence
    CS = T // 512             # distinct 512-col blocks of the rope tables

    # rmode: "all" | "qk32" | "fp32", or a 3-tuple of dtypes
    # (d_qkproj, d_attn, d_out) for bisection.
    if isinstance(rmode, tuple):
        QKD, AD, OD = rmode
    elif rmode == "all":
        QKD = AD = OD = F32R
    elif rmode == "qk32":
        QKD, AD, OD = F32, F32R, F32R
    else:
        QKD = AD = OD = F32

    nc = bacc.Bacc("TRN2", target_bir_lowering=False, debug=False,
                   num_devices=n_cores)

    xT = nc.dram_tensor("xT", [C, TOK], QKD, kind="ExternalInput").ap()
    wq = nc.dram_tensor("wq", [C, 128], QKD, kind="ExternalInput").ap()
    wk = nc.dram_tensor("wk", [C, 128], QKD, kind="ExternalInput").ap()
    wv = nc.dram_tensor("wv", [C, 128], AD, kind="ExternalInput").ap()
    wo = nc.dram_tensor("wo", [128, C], OD, kind="ExternalInput").ap()
    cosT = nc.dram_tensor("cosT", [128, T], F32, kind="ExternalInput").ap()
    sinT = nc.dram_tensor("sinT", [128, T], F32, kind="ExternalInput").ap()
    msk = nc.dram_tensor("msk", [128, 2048], F32, kind="ExternalInput").ap()
    rot = nc.dram_tensor("rot", [128, 128], AD, kind="ExternalInput").ap()
    if use_qk_bias:
        bq = nc.dram_tensor("bq", [128, 1], F32, kind="ExternalInput").ap()
        bk = nc.dram_tensor("bk", [128, 1], F32, kind="ExternalInput").ap()
    if use_v_bias:
        bv = nc.dram_tensor("bv", [128, 1], F32, kind="ExternalInput").ap()
    outp = nc.dram_tensor("outp", [TOK, C], F32, kind="ExternalOutput").ap()

    with tile.TileContext(nc) as tc:
        with ExitStack() as res:  # tensors resident through phases 1+2
            persist = res.enter_context(tc.tile_pool(name="persist", bufs=1))
            qT = persist.tile([128, TOK], AD)
            kT = persist.tile([128, TOK], AD)
            vsb = persist.tile([128, NCHUNK * 130], AD)
            ones_sb = persist.tile([1, 64], AD)
            nc.vector.memset(ones_sb[:].bitcast(F32) if AD == F32R else ones_sb[:], 1.0)

            with tc.tile_pool(name="ot", bufs=1) as otpool:
                OT = otpool.tile([128, TOK], OD)

                # ---------------- phase 1: qkv projection + RoPE ----------
                with ExitStack() as p1:
                    cpool = p1.enter_context(tc.tile_pool(name="p1c", bufs=1))
                    wq_sb = cpool.tile([128, C], QKD)
                    wk_sb = cpool.tile([128, C], QKD)
                    wv_sb = cpool.tile([128, C], AD)
                    cos_sb = cpool.tile([128, T], F32)
                    sin_sb = cpool.tile([128, T], F32)
                    rot_sb = cpool.tile([128, 128], AD)
                    ident = cpool.tile([128, 128], F32)
                    make_identity(nc, ident[:])
                    for k in range(KC):
                        nc.sync.dma_start(wq_sb[:, k * 128:(k + 1) * 128],
                                          wq[k * 128:(k + 1) * 128, :])
                        nc.sync.dma_start(wk_sb[:, k * 128:(k + 1) * 128],
                                          wk[k * 128:(k + 1) * 128, :])
                        nc.sync.dma_start(wv_sb[:, k * 128:(k + 1) * 128],
                                          wv[k * 128:(k + 1) * 128, :])
                    nc.sync.dma_start(cos_sb[:], cosT[:])
                    nc.sync.dma_start(sin_sb[:], sinT[:])
                    nc.sync.dma_start(rot_sb[:], rot[:])
                    if use_qk_bias:
                        bq_sb = cpool.tile([128, 1], F32)
                        bk_sb = cpool.tile([128, 1], F32)
                        nc.sync.dma_start(bq_sb[:], bq[:])
                        nc.sync.dma_start(bk_sb[:], bk[:])
                    if use_v_bias:
                        bv_sb = cpool.tile([128, 1], F32)
                        nc.sync.dma_start(bv_sb[:], bv[:])

                    xpool = p1.enter_context(tc.tile_pool(name="xp", bufs=8))
                    xrpool = (xpool if QKD == AD else
                              p1.enter_context(tc.tile_pool(name="xrp", bufs=6)))
                    qkpsum = p1.enter_context(
                        tc.tile_pool(name="qkp", bufs=2, space="PSUM"))
                    rotpsum = p1.enter_context(
                        tc.tile_pool(name="rotp", bufs=1, space="PSUM"))
                    vpsum = p1.enter_context(
                        tc.tile_pool(name="vp", bufs=2, space="PSUM"))
                    tppsum = p1.enter_context(
                        tc.tile_pool(name="tpp", bufs=2, space="PSUM"))
                    tmp = p1.enter_context(tc.tile_pool(name="tmp", bufs=6))

                    for nb in range(NB):
                        t512 = slice(nb * 512, (nb + 1) * 512)
                        cs = slice((nb % CS) * 512, (nb % CS) * 512 + 512)
                        xc = []
                        for k in range(KC):
                            t = xpool.tile([128, 512], QKD, tag="xc")
                            nc.sync.dma_start(
                                t[:], xT[k * 128:(k + 1) * 128, t512])
                            xc.append(t)
                        if QKD == AD:
                            xcv = xc
                        else:
                            # round a second copy for the fp32r v projection
                            xcv = []
                            for k in range(KC):
                                t = xrpool.tile([128, 512], AD, tag="xcv")
                                nc.gpsimd.tensor_copy(t[:], xc[k][:])
                                xcv.append(t)
                        for w_sb, b_name, dstT in ((wq_sb, "bq", qT),
                                                   (wk_sb, "bk", kT)):
                            acc = qkpsum.tile([128, 512], F32, tag="acc")
                            for k in range(KC):
                                nc.tensor.matmul(
                                    acc[:], w_sb[:, k * 128:(k + 1) * 128],
                                    xc[k][:], start=(k == 0), stop=(k == KC - 1))
                            raw = tmp.tile([128, 512], AD, tag="ropetmp")
                            if use_qk_bias:
                                b_sb = bq_sb if b_name == "bq" else bk_sb
                                nc.vector.tensor_scalar_add(raw[:], acc[:],
                                                            b_sb[:])
                            else:
                                nc.vector.tensor_copy(raw[:], acc[:])
                            rp = rotpsum.tile([128, 512], F32, tag="rp")
                            nc.tensor.matmul(rp[:], rot_sb[:], raw[:],
                                             start=True, stop=True)
                            t1 = tmp.tile([128, 512], F32, tag="ropetmp")
                            nc.vector.tensor_mul(t1[:], raw[:], cos_sb[:, cs])
                            t2 = tmp.tile([128, 512], F32, tag="ropetmp")
                            nc.vector.tensor_mul(t2[:], rp[:], sin_sb[:, cs])
                            nc.vector.tensor_add(dstT[:, t512], t1[:], t2[:])
                        # v: project transposed (N=512 streams), then PE-
                        # transpose back to [tokens, 128] chunks
                        vacc = vpsum.tile([128, 512], F32, tag="vacc")
                        for k in range(KC):
                            nc.tensor.matmul(
                                vacc[:], wv_sb[:, k * 128:(k + 1) * 128],
                                xcv[k][:], start=(k == 0), stop=(k == KC - 1))
                        vraw = tmp.tile([128, 512], F32, tag="vraw", bufs=2)
                        if use_v_bias:
                            nc.vector.tensor_scalar_add(vraw[:], vacc[:],
                                                        bv_sb[:])
                        else:
                            nc.vector.tensor_copy(vraw[:], vacc[:])
                        for sub in range(4):
                            tp = tppsum.tile([128, 128], F32, tag="tp")
                            nc.tensor.transpose(
                                tp[:], vraw[:, sub * 128:(sub + 1) * 128],
                                ident[:])
                            base = (nb * 4 + sub) * 130
                            nc.vector.tensor_copy(vsb[:, base:base + 64],
                                                  tp[:, 0:64])
                            nc.vector.tensor_copy(vsb[:, base + 65:base + 129],
                                                  tp[:, 64:128])
                            nc.vector.memset(vsb[:, base + 64:base + 65].bitcast(F32) if AD == F32R else vsb[:, base + 64:base + 65], 1.0)
                            nc.vector.memset(vsb[:, base + 129:base + 130].bitcast(F32) if AD == F32R else vsb[:, base + 129:base + 130], 1.0)

                # ---------------- phase 2: causal attention ---------------
                with ExitStack() as p2:
                    mpool = p2.enter_context(tc.tile_pool(name="mp", bufs=1))
                    msk_sb = mpool.tile([128, 2048], F32)
                    nc.sync.dma_start(msk_sb[:], msk[:])
                    spsum = p2.enter_context(
                        tc.tile_pool(name="sp", bufs=3, space="PSUM"))
                    popsum = p2.enter_context(
                        tc.tile_pool(name="pop", bufs=2, space="PSUM"))
                    bcpsum = p2.enter_context(
                        tc.tile_pool(name="bcp", bufs=2, space="PSUM"))
                    ppool = p2.enter_context(tc.tile_pool(name="pp", bufs=3))
                    dpool = p2.enter_context(tc.tile_pool(name="dp", bufs=2))
                    bspool = p2.enter_context(tc.tile_pool(name="bs", bufs=2))

                    for b in range(B):
                        t0 = b * T
                        c0 = b * (T // 128)
                        for hp in range(HPC):
                            hs = slice(hp * 64, (hp + 1) * 64)
                            for qb in range(QB):
                                q512 = slice(t0 + qb * 512, t0 + (qb + 1) * 512)
                                po = popsum.tile([65, 512], F32, tag="po")
                                ns = (qb + 1) * 4
                                for si in range(ns):
                                    S = spsum.tile([128, 512], F32, tag="S")
                                    s0 = t0 + si * 128
                                    nc.tensor.matmul(
                                        S[:], kT[hs, s0:s0 + 128],
                                        qT[hs, q512], start=True, stop=True)
                                    P = ppool.tile([128, 512], AD, tag="P")
                                    nc.scalar.activation(P[:], S[:], AF.Exp)
                                    off = si * 128 - qb * 512
                                    if off >= 0:
                                        vi = off // 128
                                        nc.vector.tensor_mul(
                                            P[:], P[:],
                                            msk_sb[:, vi * 512:(vi + 1) * 512])
                                    vbase = (c0 + si) * 130 + hp * 65
                                    nc.tensor.matmul(
                                        po[:], vsb[:, vbase:vbase + 65], P[:],
                                        start=(si == 0), stop=(si == ns - 1))
                                den32 = dpool.tile([1, 512], F32, tag="den32")
                                nc.vector.reciprocal(den32[:], po[64:65, :])
                                den = dpool.tile([1, 512], AD, tag="den")
                                nc.vector.tensor_copy(den[:], den32[:])
                                bc = bcpsum.tile([64, 512], F32, tag="bc")
                                nc.tensor.matmul(bc[:], ones_sb[:], den[:],
                                                 start=True, stop=True)
                                bcs = bspool.tile([64, 512], F32, tag="bcs")
                                nc.vector.tensor_copy(bcs[:], bc[:])
                                nc.vector.tensor_mul(
                                    OT[hs, q512], po[0:64, :], bcs[:])

                # ---------------- phase 3: output projection --------------
                with ExitStack() as p3:
                    wpool = p3.enter_context(tc.tile_pool(name="wop", bufs=1))
                    wo_sb = wpool.tile([128, C], OD)
                    nc.sync.dma_start(wo_sb[:], wo[:])
                    opsum = p3.enter_context(
                        tc.tile_pool(name="op", bufs=4, space="PSUM"))
                    ostage = p3.enter_context(tc.tile_pool(name="os", bufs=4))
                    for ci in range(NCHUNK):
                        for n2 in range(C // 512):
                            pacc = opsum.tile([128, 512], F32, tag="pacc")
                            nc.tensor.matmul(
                                pacc[:], OT[:, ci * 128:(ci + 1) * 128],
                                wo_sb[:, n2 * 512:(n2 + 1) * 512],
                                start=True, stop=True)
                            osb = ostage.tile([128, 512], F32, tag="osb")
                            if (ci + n2) % 2 == 0:
                                nc.scalar.activation(osb[:], pacc[:], AF.Copy)
                            else:
                                nc.vector.tensor_copy(osb[:], pacc[:])
                            nc.sync.dma_start(
                                outp[ci * 128:(ci + 1) * 128,
                                     n2 * 512:(n2 + 1) * 512], osb[:])

    nc.compile()
    return nc


def make_rope_tables(T, dtype=np.float32):
    j = np.arange(32, dtype=np.float32)
    inv_freq = (1.0 / (ROPE_BASE ** (2.0 * j / D))).astype(np.float32)
    t = np.arange(T, dtype=np.float32)
    freqs = t[None, :] * inv_freq[:, None]          # [32, T]
    half = np.concatenate([freqs, freqs], axis=0)   # [64, T]
    cosT = np.cos(half).astype(dtype)
    sinT = np.sin(half).astype(dtype)
    return (np.concatenate([cosT, cosT], axis=0),   # [128, T] (2 heads)
            np.concatenate([sinT, sinT], axis=0))


def make_rot_matrix():
    """lhsT R [128,128] s.t. (R.T @ x)[m] = rotate_half(x)[m] per 64-row head."""
    R = np.zeros((128, 128), dtype=np.float32)
    for hb in (0, 64):
        for m in range(32):
            R[hb + m + 32, hb + m] = -1.0
            R[hb + m, hb + m + 32] = 1.0
    return R


def make_masks():
    """[128, 4*512]: variant vi valid where s + vi*128 <= q."""
    s = np.arange(128)[:, None]
    q = np.arange(512)[None, :]
    blocks = [(s + vi * 128 <= q).astype(np.float32) for vi in range(4)]
    return np.concatenate(blocks, axis=1)


def prep_in_maps(x, W_qkv, b_qkv, W_out, B, T, use_qk_bias, use_v_bias,
                 n_cores=N_CORES):
    TOK = B * T
    xTm = np.ascontiguousarray(x.reshape(TOK, C).T)
    cosT, sinT = make_rope_tables(T)
    msk = make_masks()
    rot = make_rot_matrix()
    in_maps = []
    for c in range(n_cores):
        h0 = c * HPC
        cols = slice(h0 * D, (h0 + HPC) * D)        # 128 head-dim columns
        m = {
            "xT": xTm,
            "wq": np.ascontiguousarray(W_qkv[:, cols]),
            "wk": np.ascontiguousarray(W_qkv[:, C:][:, cols]),
            "wv": np.ascontiguousarray(W_qkv[:, 2 * C:][:, cols]),
            "wo": np.ascontiguousarray(W_out[cols, :]),
            "cosT": cosT, "sinT": sinT, "msk": msk, "rot": rot,
        }
        if use_qk_bias:
            m["bq"] = np.ascontiguousarray(b_qkv[cols]).reshape(128, 1)
            m["bk"] = np.ascontiguousarray(b_qkv[C:][cols]).reshape(128, 1)
        if use_v_bias:
            m["bv"] = np.ascontiguousarray(b_qkv[2 * C:][cols]).reshape(128, 1)
        in_maps.append(m)
    return in_maps


_CACHE = {}


def _get_program(key):
    if key not in _CACHE:
        B, T, use_qk_bias, use_v_bias = key
        _CACHE[key] = build_program(B, T, use_qk_bias, use_v_bias)
    return _CACHE[key]


def kernel(x, W_qkv, b_qkv, W_out, b_out):
    x = np.asarray(x, dtype=np.float32)
    W_qkv = np.asarray(W_qkv, dtype=np.float32)
    b_qkv = np.asarray(b_qkv, dtype=np.float32)
    W_out = np.asarray(W_out, dtype=np.float32)
    b_out = np.asarray(b_out, dtype=np.float32)
    B, T, _ = x.shape
    use_qk_bias = bool(np.any(b_qkv[:2 * C]))
    use_v_bias = bool(np.any(b_qkv[2 * C:]))
    nc = _get_program((B, T, use_qk_bias, use_v_bias))
    in_maps = prep_in_maps(x, W_qkv, b_qkv, W_out, B, T,
                           use_qk_bias, use_v_bias)
    res = run_bass_kernel_spmd(nc, in_maps, list(range(N_CORES)))
    acc = res.results[0]["outp"].astype(np.float32)
    for c in range(1, N_CORES):
        acc = acc + res.results[c]["outp"]
    acc = acc + b_out[None, :]
    return acc.reshape(B, T, C)
